# revision 11
# baseline (speedup 1.0000x reference)
"""Multi-head attention block (B=8, S=1024, H=768, 12 heads x 64) on 8 TRN2 cores.

Sharding: pure data-parallel - one batch element per NeuronCore, no collectives.
All matmuls run as fp8e4m3 DoubleRow (2 contraction rows per PE pass):
  - QKV projections pair-pack the H=768 contraction (3 x K=256 matmuls),
    with host-side weight layouts.
  - Scores run DoubleRow over dh=64 (32 partitions x 2 slots per head) via a
    host-side column permutation of Wq/Wk so Q^T/K^T land in [32,2,S] layout.
  - Context (attn @ V) pair-packs adjacent key chunks j via strided exp/V
    tiles [128, 8, *].
Softmax: exp(score/8 - 2) on ACT (shift keeps fp8e4m3 in range; softmax is
shift-invariant), row sums via tiny DoubleRow matmuls against a 0.5-column,
normalization + residual fused in one DVE op per (head, m) with row-sum
accumulation feeding the LayerNorm mean. LayerNorm in bf16, y output bf16.
"""

import sys

sys.path.insert(0, "/opt/trn_rl_repo")

import numpy as np
import ml_dtypes
from contextlib import ExitStack

import concourse.bacc as bacc
import concourse.tile as tile
from concourse import mybir
from concourse import bass_utils

AF = mybir.ActivationFunctionType
ALU = mybir.AluOpType
AX = mybir.AxisListType
PM = mybir.MatmulPerfMode

F32 = mybir.dt.float32
F8 = mybir.dt.float8e4
BF16 = mybir.dt.bfloat16

NPF8 = ml_dtypes.float8_e4m3
NPBF16 = ml_dtypes.bfloat16

B, S, H, NH, DH = 8, 1024, 768, 12, 64
P = 128
EPS = 1e-6
ESHIFT = -2.0  # exp(s/8 + ESHIFT): keeps exp outputs within fp8e4m3 range

_cache = {}


def _build(affine: bool, repeats: int = 1):
    nc = bacc.Bacc("TRN2", target_bir_lowering=False, debug=False)

    xp_d = nc.dram_tensor("xp", [P, 3, 2, S], F8, kind="ExternalInput")
    xn2_d = nc.dram_tensor("xn2", [S, H], BF16, kind="ExternalInput")
    wq_d = nc.dram_tensor("wq", [P, 3, 2, H], F8, kind="ExternalInput")
    wk_d = nc.dram_tensor("wk", [P, 3, 2, H], F8, kind="ExternalInput")
    wv_d = nc.dram_tensor("wv", [P, 3, 2, H], F8, kind="ExternalInput")
    bqp_d = nc.dram_tensor("bqp", [P, 6], F32, kind="ExternalInput")
    bkp_d = nc.dram_tensor("bkp", [P, 6], F32, kind="ExternalInput")
    if affine:
        gam_d = nc.dram_tensor("gamb", [P, H], BF16, kind="ExternalInput")
        bet_d = nc.dram_tensor("betb", [P, H], BF16, kind="ExternalInput")
    y_d = nc.dram_tensor("y", [S, H], BF16, kind="ExternalOutput")

    dram = dict(xp_d=xp_d, xn2_d=xn2_d, wq_d=wq_d, wk_d=wk_d, wv_d=wv_d,
                bqp_d=bqp_d, bkp_d=bkp_d, y_d=y_d,
                gam_d=gam_d if affine else None,
                bet_d=bet_d if affine else None)
    with ExitStack() as stk:
        tc = stk.enter_context(tile.TileContext(nc))
        for rep in range(repeats):
            if rep:
                tc.strict_bb_all_engine_barrier()
            _emit_once(nc, tc, dram, affine, rep)
    nc.compile()
    return nc


def _emit_once(nc, tc, dram, affine, rep):
    xp_d, xn2_d, y_d = dram["xp_d"], dram["xn2_d"], dram["y_d"]
    wq_d, wk_d, wv_d = dram["wq_d"], dram["wk_d"], dram["wv_d"]
    bqp_d, bkp_d = dram["bqp_d"], dram["bkp_d"]
    gam_d, bet_d = dram["gam_d"], dram["bet_d"]

    with ExitStack() as stk:
        lp = stk.enter_context(tc.tile_pool(name=f"long{rep}", bufs=1))
        ap = stk.enter_context(tc.tile_pool(name=f"attn{rep}", bufs=1))
        ps = stk.enter_context(tc.tile_pool(name=f"ps{rep}", bufs=1, space="PSUM"))

        # ---- PSUM tags (created in order for bank alignment) ----
        # scA [128,2048] f32 = banks 0-3; scB [128,1024] = banks 4-5;
        # pc 2x[128,128] + pr 2x[128,8] live in bank 6.
        _n = [0]

        def psA():
            _n[0] += 1
            return ps.tile([P, 2048], F32, tag="scA", bufs=1,
                           name=f"psA{_n[0]}")

        def psB():
            _n[0] += 1
            return ps.tile([P, 1024], F32, tag="scB", bufs=1,
                           name=f"psB{_n[0]}")

        # ---- loads ----
        xp = lp.tile([P, 3, 2, S], F8, tag="xp")
        nc.sync.dma_start(xp, xp_d[:, :, :, :])
        W = {}
        for nm, d in (("q", wq_d), ("k", wk_d), ("v", wv_d)):
            t = lp.tile([P, 3, 2, H], F8, tag=f"w{nm}", name=f"w{nm}")
            nc.sync.dma_start(t, d[:, :, :, :])
            W[nm] = t
        bqp = lp.tile([P, 6], F32, tag="bqp")
        nc.sync.dma_start(bqp, bqp_d[:, :])
        bkp = lp.tile([P, 6], F32, tag="bkp")
        nc.sync.dma_start(bkp, bkp_d[:, :])
        xn2 = []
        for m in range(8):
            t = lp.tile([P, H], BF16, tag=f"xn{m}", name=f"xn{m}")
            nc.sync.dma_start(t, xn2_d[m * P:(m + 1) * P, :])
            xn2.append(t)
        if affine:
            gamb = lp.tile([P, H], BF16, tag="gamb")
            nc.sync.dma_start(gamb, gam_d[:, :])
            betb = lp.tile([P, H], BF16, tag="betb")
            nc.sync.dma_start(betb, bet_d[:, :])

        halfc = lp.tile([P, 2], F8, tag="halfc")
        nc.vector.memset(halfc, 0.5)
        ebias = lp.tile([P, 1], F32, tag="ebias")
        nc.vector.memset(ebias, ESHIFT)
        epsb = lp.tile([P, 1], F32, tag="epsb")
        nc.vector.memset(epsb, EPS)

        # ---- long-lived SBUF tensors ----
        QT = [lp.tile([P, 2, S], F8, tag=f"qt{t}", name=f"qt{t}")
              for t in range(3)]
        KT = [lp.tile([P, 2, S], F8, tag=f"kt{t}", name=f"kt{t}")
              for t in range(3)]
        # matmul APs can only start at partition 0/32/64, so heads living on
        # partitions 96-127 (h%4==3) get DMA-copied to legal bases here
        QX = lp.tile([P, 2, S], F8, tag="qx")
        KX = lp.tile([P, 2, S], F8, tag="kx")
        V8 = lp.tile([P, 8, H], F8, tag="v8")
        E8 = [lp.tile([P, 8, S], F8, tag=f"e8_{i}", name=f"e8_{i}")
              for i in range(3)]
        Y = [lp.tile([P, H], BF16, tag=f"y{m}", name=f"yt{m}")
             for m in range(8)]
        RS = [lp.tile([P, 16], F32, tag=f"rs{m}", name=f"rs{m}")
              for m in range(8)]

        # ---- projections ----
        def proj_qk(w_sb, b_sb, out_t, t, i_s):
            pt = psA() if (t * 2 + i_s) % 2 == 0 else psB()
            c0 = t * 256 + i_s * 128
            for ns in (0, 512):
                for g in range(3):
                    nc.tensor.matmul(
                        pt[:, ns:ns + 512],
                        lhsT=w_sb[:, g, :, c0:c0 + 128],
                        rhs=xp[:, g, :, ns:ns + 512],
                        start=(g == 0), stop=(g == 2),
                        perf_mode=PM.DoubleRow,
                    )
            nc.vector.tensor_scalar(
                out=out_t[:, i_s, :], in0=pt[:, 0:1024],
                scalar1=b_sb[:, t * 2 + i_s:t * 2 + i_s + 1], scalar2=None,
                op0=ALU.add,
            )

        def proj_v(m):
            pt = psA() if m % 2 == 0 else psB()
            for c0, nn in ((0, 512), (512, 256)):
                for g in range(3):
                    nc.tensor.matmul(
                        pt[:, c0:c0 + nn],
                        lhsT=xp[:, g, :, m * P:(m + 1) * P],
                        rhs=W["v"][:, g, :, c0:c0 + nn],
                        start=(g == 0), stop=(g == 2),
                        perf_mode=PM.DoubleRow,
                    )
            nc.vector.tensor_copy(V8[:, m, :], pt[:, 0:H])

        # ---- attention emitters ----
        # per head: 5 exp units over psum slots A(j0,j1) B(j2) A(j3,j4) B(j5) A(j6,j7)
        UNITS = ((0, 2), (2, 1), (3, 2), (5, 1), (6, 2))

        def emit_scores(h):
            t, gh = divmod(h, 4)
            sl = h % 3
            if gh == 3:
                qt, kt, bp = QX, KX, 32 * t
            else:
                qt, kt, bp = QT[t], KT[t], 32 * gh
            for j0, nj in UNITS:
                pt = psA() if nj == 2 else psB()
                for jj in range(nj):
                    j = j0 + jj
                    for ns in (0, 512):
                        nc.tensor.matmul(
                            pt[:, jj * 1024 + ns:jj * 1024 + ns + 512],
                            lhsT=kt[bp:bp + 32, :, j * P:(j + 1) * P],
                            rhs=qt[bp:bp + 32, :, ns:ns + 512],
                            start=True, stop=True,
                            perf_mode=PM.DoubleRow,
                        )
                nc.scalar.activation(
                    E8[sl][:, j0:j0 + nj, :], pt[:, 0:nj * 1024],
                    AF.Exp, scale=0.125, bias=ebias[:, 0:1],
                )

        def emit_ctx(h):
            sl = h % 3
            prt = ps.tile([P, 128], F32, tag="small", bufs=2, name=f"pr{h}")
            pr = prt[:, 0:8]
            for m in range(8):
                for jp in range(4):
                    nc.tensor.matmul(
                        pr[:, m:m + 1],
                        lhsT=E8[sl][:, jp * 2:jp * 2 + 2, m * P:(m + 1) * P],
                        rhs=halfc[:, :].rearrange("p (i o) -> p i o", o=1),
                        start=(jp == 0), stop=(jp == 3),
                        perf_mode=PM.DoubleRow,
                    )
            rinv = ap.tile([P, 8], F32, tag="rinv", bufs=2, name=f"rinv{h}")
            nc.vector.reciprocal(rinv, pr)
            for m in range(8):
                pc = ps.tile([P, 128], F32, tag="small", bufs=2,
                             name=f"pc{h}_{m}")
                for jp in range(4):
                    nc.tensor.matmul(
                        pc[:, 0:64],
                        lhsT=E8[sl][:, jp * 2:jp * 2 + 2, m * P:(m + 1) * P],
                        rhs=V8[:, jp * 2:jp * 2 + 2, h * 64:(h + 1) * 64],
                        start=(jp == 0), stop=(jp == 3),
                        perf_mode=PM.DoubleRow,
                    )
                nc.vector.scalar_tensor_tensor(
                    out=Y[m][:, h * 64:(h + 1) * 64], in0=pc[:, 0:64],
                    scalar=rinv[:, m:m + 1], in1=xn2[m][:, h * 64:(h + 1) * 64],
                    op0=ALU.mult, op1=ALU.add,
                    accum_out=RS[m][:, h:h + 1],
                )

        def emit_ln(m):
            srow = ap.tile([P, 1], F32, tag="srow", bufs=3, name=f"srow{m}")
            nc.vector.tensor_reduce(out=srow, in_=RS[m][:, 0:NH], axis=AX.X,
                                    op=ALU.add)
            nmu = ap.tile([P, 1], F32, tag="nmu", bufs=3, name=f"nmu{m}")
            nc.vector.tensor_scalar(out=nmu, in0=srow, scalar1=-1.0 / H,
                                    scalar2=None, op0=ALU.mult)
            nc.gpsimd.tensor_scalar(out=Y[m], in0=Y[m], scalar1=nmu[:, 0:1],
                                    scalar2=None, op0=ALU.add)
            ysq = ap.tile([P, H], BF16, tag="ysq", bufs=2, name=f"ysq{m}")
            ss = ap.tile([P, 1], F32, tag="ss", bufs=3, name=f"ss{m}")
            nc.vector.scalar_tensor_tensor(
                out=ysq, in0=Y[m], scalar=1.0, in1=Y[m],
                op0=ALU.mult, op1=ALU.mult, accum_out=ss[:, 0:1],
            )
            sd = ap.tile([P, 1], F32, tag="sd", bufs=3, name=f"sd{m}")
            nc.scalar.activation(sd, ss, AF.Sqrt, scale=1.0 / H,
                                 bias=epsb[:, 0:1])
            rstd = ap.tile([P, 1], F32, tag="rstd", bufs=3, name=f"rstd{m}")
            nc.vector.reciprocal(rstd, sd)
            nc.gpsimd.tensor_scalar(out=Y[m], in0=Y[m], scalar1=rstd[:, 0:1],
                                    scalar2=None, op0=ALU.mult)
            if affine:
                nc.gpsimd.tensor_tensor(out=Y[m], in0=Y[m], in1=gamb,
                                        op=ALU.mult)
                nc.gpsimd.tensor_tensor(out=Y[m], in0=Y[m], in1=betb,
                                        op=ALU.add)
            nc.sync.dma_start(y_d[m * P:(m + 1) * P, :], Y[m])

        # ---- schedule ----
        proj_qk(W["q"], bqp, QT[0], 0, 0)
        proj_qk(W["q"], bqp, QT[0], 0, 1)
        proj_qk(W["k"], bkp, KT[0], 0, 0)
        proj_qk(W["k"], bkp, KT[0], 0, 1)
        nc.sync.dma_start(QX[0:32, :, :], QT[0][96:P, :, :])
        nc.sync.dma_start(KX[0:32, :, :], KT[0][96:P, :, :])
        emit_scores(0)
        for t in (1, 2):
            proj_qk(W["q"], bqp, QT[t], t, 0)
            proj_qk(W["q"], bqp, QT[t], t, 1)
            proj_qk(W["k"], bkp, KT[t], t, 0)
            proj_qk(W["k"], bkp, KT[t], t, 1)
            nc.sync.dma_start(QX[32 * t:32 * t + 32, :, :],
                              QT[t][96:P, :, :])
            nc.sync.dma_start(KX[32 * t:32 * t + 32, :, :],
                              KT[t][96:P, :, :])
        emit_scores(1)
        for m in range(8):
            proj_v(m)
        emit_scores(2)
        emit_ctx(0)
        for h in range(1, NH):
            if h + 2 < NH:
                emit_scores(h + 2)
            emit_ctx(h)
        for m in range(8):
            emit_ln(m)


def _get_nc(affine: bool):
    if affine not in _cache:
        _cache[affine] = _build(affine)
    return _cache[affine]


def _is_affine(inputs):
    gam = np.asarray(inputs["ln_gamma"], dtype=np.float32)
    bet = np.asarray(inputs["ln_beta"], dtype=np.float32)
    return not (np.all(gam == 1.0) and np.all(bet == 0.0))


def _perm_cols():
    # c' = t*256 + i*128 + m  ->  orig col 64*(4t + m//32) + 32*i + (m%32)
    cp = np.arange(H)
    t, r = np.divmod(cp, 256)
    i, m = np.divmod(r, 128)
    return 64 * (4 * t + m // 32) + 32 * i + (m % 32)


def _pair_rows(a):
    """[768, N] -> [128, 3, 2, N] with (p, g, i) <-> row 256g + 128i + p."""
    n = a.shape[1]
    return np.ascontiguousarray(
        a.reshape(3, 2, P, n).transpose(2, 0, 1, 3))


def make_in_maps(inputs):
    x = np.asarray(inputs["x"], dtype=np.float32)
    Wq = np.asarray(inputs["Wq"], dtype=np.float32)
    Wk = np.asarray(inputs["Wk"], dtype=np.float32)
    Wv = np.asarray(inputs["Wv"], dtype=np.float32)
    bq = np.asarray(inputs["bq"], dtype=np.float32)
    bk = np.asarray(inputs["bk"], dtype=np.float32)
    bv = np.asarray(inputs["bv"], dtype=np.float32)
    affine = _is_affine(inputs)

    perm = _perm_cols()
    wq8 = _pair_rows(Wq[:, perm]).astype(NPF8)
    wk8 = _pair_rows(Wk[:, perm]).astype(NPF8)
    wv8 = _pair_rows(Wv).astype(NPF8)

    # bias in output-partition layout: bqp[q, t*2+i] = bq[perm[t*256+i*128+q]]
    bqp = np.empty((P, 6), np.float32)
    bkp = np.empty((P, 6), np.float32)
    for t in range(3):
        for i in range(2):
            cols = perm[t * 256 + i * 128: t * 256 + i * 128 + P]
            bqp[:, t * 2 + i] = bq[cols]
            bkp[:, t * 2 + i] = bk[cols]

    in_maps = []
    for b in range(B):
        xb = x[b]
        im = {
            "xp": _pair_rows(np.ascontiguousarray(xb.T)).astype(NPF8),
            "xn2": (xb + 2.0 * bv).astype(NPBF16),
            "wq": wq8, "wk": wk8, "wv": wv8,
            "bqp": bqp, "bkp": bkp,
        }
        if affine:
            im["gamb"] = np.broadcast_to(
                np.asarray(inputs["ln_gamma"], np.float32), (P, H)
            ).astype(NPBF16)
            im["betb"] = np.broadcast_to(
                np.asarray(inputs["ln_beta"], np.float32), (P, H)
            ).astype(NPBF16)
        in_maps.append(im)
    return in_maps


def run(inputs, trace=False):
    nc = _get_nc(_is_affine(inputs))
    in_maps = make_in_maps(inputs)
    res = bass_utils.run_bass_kernel_spmd(
        nc, in_maps, core_ids=list(range(B)), trace=trace
    )
    out = np.stack(
        [r["y"].astype(np.float32) for r in res.results], axis=0)
    return out, res


def kernel(**inputs) -> np.ndarray:
    out, _ = run(inputs, trace=False)
    return out


# revision 61
# speedup vs baseline: 3.0543x; 3.0543x over previous
"""Multi-head attention block (B=8, S=1024, H=768, 12 heads x 64) on 8 TRN2 cores.

Sharding: pure data-parallel - one batch element per NeuronCore, no collectives.
All matmuls run as fp8e4m3 DoubleRow (2 contraction rows per PE pass):
  - QKV projections pair-pack the H=768 contraction (3 x K=256 matmuls),
    with host-side weight layouts.
  - Scores run DoubleRow over dh=64 (32 partitions x 2 slots per head) via a
    host-side column permutation of Wq/Wk so Q^T/K^T land in [32,2,S] layout.
  - Context (attn @ V) pair-packs adjacent key chunks j via strided exp/V
    tiles [128, 8, *].
Softmax: exp(score/8 - 2) on ACT (shift keeps fp8e4m3 in range; softmax is
shift-invariant), row sums via tiny DoubleRow matmuls against a 0.5-column,
normalization + residual fused in one DVE op per (head, m) with row-sum
accumulation feeding the LayerNorm mean. LayerNorm in bf16, y output bf16.
"""

import sys

sys.path.insert(0, "/opt/trn_rl_repo")

import numpy as np
import ml_dtypes
from contextlib import ExitStack

import concourse.bacc as bacc
import concourse.tile as tile
from concourse import mybir
from concourse import bass_utils

AF = mybir.ActivationFunctionType
ALU = mybir.AluOpType
AX = mybir.AxisListType
PM = mybir.MatmulPerfMode

F32 = mybir.dt.float32
F8 = mybir.dt.float8e4
BF16 = mybir.dt.bfloat16

NPF8 = ml_dtypes.float8_e4m3
NPBF16 = ml_dtypes.bfloat16

B, S, H, NH, DH = 8, 1024, 768, 12, 64
P = 128
EPS = 1e-6
ESHIFT = -2.0  # exp(s/8 + ESHIFT): keeps exp outputs within fp8e4m3 range

# heads whose query-half-1 softmax exp runs on DVE via the Schraudolph int16
# bit trick (bits of bf16(exp(s/8-2)) ~= round(s*SC1 + SC2)): ctx for query
# chunks m>=4 only reads half-1 scores, so the fp8/bf16 split is clean per m.
OFF_HEADS = (1, 3, 5, 7, 9, 11)
SC1 = 0.125 * 1.4426950408889634 * 128
SC2 = (127.0 - 2 * 1.4426950408889634) * 128 - 7.5

_cache = {}


def _build(affine: bool, repeats: int = 1):
    nc = bacc.Bacc("TRN2", target_bir_lowering=False, debug=False)

    xp_d = nc.dram_tensor("xp", [P, 3, 2, S], F8, kind="ExternalInput")
    xn2_d = nc.dram_tensor("xn2", [S, H], BF16, kind="ExternalInput")
    wq_d = nc.dram_tensor("wq", [P, 3, 2, H], F8, kind="ExternalInput")
    wk_d = nc.dram_tensor("wk", [P, 3, 2, H], F8, kind="ExternalInput")
    wv_d = nc.dram_tensor("wv", [P, 3, 2, H], F8, kind="ExternalInput")
    bqp_d = nc.dram_tensor("bqp", [P, 6], F32, kind="ExternalInput")
    bkp_d = nc.dram_tensor("bkp", [P, 6], F32, kind="ExternalInput")
    if affine:
        gam_d = nc.dram_tensor("gamb", [P, H], BF16, kind="ExternalInput")
        bet_d = nc.dram_tensor("betb", [P, H], BF16, kind="ExternalInput")
    y_d = nc.dram_tensor("y", [S, H], BF16, kind="ExternalOutput")

    dram = dict(xp_d=xp_d, xn2_d=xn2_d, wq_d=wq_d, wk_d=wk_d, wv_d=wv_d,
                bqp_d=bqp_d, bkp_d=bkp_d, y_d=y_d,
                gam_d=gam_d if affine else None,
                bet_d=bet_d if affine else None)
    with ExitStack() as stk:
        tc = stk.enter_context(tile.TileContext(nc))
        for rep in range(repeats):
            if rep:
                tc.strict_bb_all_engine_barrier()
            _emit_once(nc, tc, dram, affine, rep)
    nc.compile()
    return nc


def _emit_once(nc, tc, dram, affine, rep):
    xp_d, xn2_d, y_d = dram["xp_d"], dram["xn2_d"], dram["y_d"]
    wq_d, wk_d, wv_d = dram["wq_d"], dram["wk_d"], dram["wv_d"]
    bqp_d, bkp_d = dram["bqp_d"], dram["bkp_d"]
    gam_d, bet_d = dram["gam_d"], dram["bet_d"]

    with ExitStack() as stk:
        lp = stk.enter_context(tc.tile_pool(name=f"long{rep}", bufs=1))
        ap = stk.enter_context(tc.tile_pool(name=f"attn{rep}", bufs=1))
        ps = stk.enter_context(tc.tile_pool(name=f"ps{rep}", bufs=1, space="PSUM"))

        # ---- PSUM tags (created in order for bank alignment) ----
        # scA [128,2048] f32 = banks 0-3; scB [128,1024] = banks 4-5;
        # pc 2x[128,128] + pr 2x[128,8] live in bank 6.
        _n = [0]

        def psAB():
            # strict two-slot alternation: every score/proj psum allocation
            # ping-pongs between two 3-bank slots, so ACT never sees two
            # consecutive exp units on the same slot
            _n[0] += 1
            tag = "scA" if _n[0] % 2 == 0 else "scB"
            return ps.tile([P, 1536], F32, tag=tag, bufs=1,
                           name=f"ps{_n[0]}")

        # ---- loads: critical tensors issued in parallel across engine DMA
        # queues (SP-only issue serializes at ~565ns each)
        xp = lp.tile([P, 3, 2, S], F8, tag="xp")
        W = {}
        for nm in ("q", "k", "v"):
            W[nm] = lp.tile([P, 3, 2, H], F8, tag=f"w{nm}", name=f"w{nm}")
        bqp = lp.tile([P, 6], F32, tag="bqp")
        bkp = lp.tile([P, 6], F32, tag="bkp")

        nc.sync.dma_start(W["q"][:, 0, :, 0:256], wq_d[:, 0, :, 0:256])
        nc.gpsimd.dma_start(W["k"][:, 0, :, 0:256], wk_d[:, 0, :, 0:256])
        nc.scalar.dma_start(xp[:, 0, :, :], xp_d[:, 0, :, :])
        nc.gpsimd.dma_start(xp[:, 1, :, :], xp_d[:, 1, :, :])
        nc.scalar.dma_start(xp[:, 2, :, :], xp_d[:, 2, :, :])
        nc.sync.dma_start(W["q"][:, 1:3, :, 0:256], wq_d[:, 1:3, :, 0:256])
        nc.gpsimd.dma_start(W["k"][:, 1:3, :, 0:256], wk_d[:, 1:3, :, 0:256])
        nc.sync.dma_start(bqp, bqp_d[:, :])
        nc.sync.dma_start(bkp, bkp_d[:, :])
        for nm in ("q", "k"):
            nc.sync.dma_start(W[nm][:, :, :, 256:768],
                              dram[f"w{nm}_d"][:, :, :, 256:768])
        nc.sync.dma_start(W["v"], wv_d[:, :, :, :])
        xn2 = []
        for m in range(8):
            t = lp.tile([P, H], BF16, tag=f"xn{m}", name=f"xn{m}")
            nc.sync.dma_start(t, xn2_d[m * P:(m + 1) * P, :])
            xn2.append(t)
        if affine:
            gamb = lp.tile([P, H], BF16, tag="gamb")
            nc.sync.dma_start(gamb, gam_d[:, :])
            betb = lp.tile([P, H], BF16, tag="betb")
            nc.sync.dma_start(betb, bet_d[:, :])

        halfc = lp.tile([P, 2], F8, tag="halfc")
        nc.vector.memset(halfc, 0.5)
        halfcb = lp.tile([P, 2], BF16, tag="halfcb")
        nc.vector.memset(halfcb, 0.5)
        ebias = lp.tile([P, 1], F32, tag="ebias")
        nc.vector.memset(ebias, ESHIFT)
        epsb = lp.tile([P, 1], F32, tag="epsb")
        nc.vector.memset(epsb, EPS)

        # ---- long-lived SBUF tensors ----
        QT = [lp.tile([P, 2, S], F8, tag=f"qt{t}", name=f"qt{t}")
              for t in range(3)]
        KT = [lp.tile([P, 2, S], F8, tag=f"kt{t}", name=f"kt{t}")
              for t in range(3)]
        # matmul APs can only start at partition 0/32/64, so heads living on
        # partitions 96-127 (h%4==3) get DMA-copied to legal bases here
        QX = lp.tile([P, 2, S], F8, tag="qx")
        KX = lp.tile([P, 2, S], F8, tag="kx")
        V8 = lp.tile([P, 8, H], F8, tag="v8")
        NSLOT = 5  # E8 depth; scores run 4 heads ahead of ctx
        E8 = [lp.tile([P, 8, S], F8, tag=f"e8_{i}", name=f"e8_{i}")
              for i in range(NSLOT)]
        # E8b holds only query-half-1 (cols 512:1024 of each j) in bf16
        E8b = [lp.tile([P, 8, 512], BF16, tag=f"e8b_{i}", name=f"e8b_{i}")
               for i in range(3)]

        def e8b_slot(h):
            return E8b[(h // 2) % 3]
        Y = [lp.tile([P, H], BF16, tag=f"y{m}", name=f"yt{m}")
             for m in range(8)]
        RS = [lp.tile([P, 16], F32, tag=f"rs{m}", name=f"rs{m}")
              for m in range(8)]

        # ---- projections ----
        _stg = [0]

        def stage_slot(name):
            # 1-bank staging psums for mid-stream projections, away from the
            # score-slot rotation
            _stg[0] += 1
            tag = "vps" if _stg[0] % 2 == 0 else "small"
            return ps.tile([P, 512], F32, tag=tag, bufs=1, name=name)

        def proj_qk(w_sb, b_sb, out_t, t, i_s, staged=False):
            c0 = t * 256 + i_s * 128
            bias = b_sb[:, t * 2 + i_s:t * 2 + i_s + 1]
            if not staged:
                pt = psAB()
                for ns in (0, 512):
                    for g in range(3):
                        nc.tensor.matmul(
                            pt[:, ns:ns + 512],
                            lhsT=w_sb[:, g, :, c0:c0 + 128],
                            rhs=xp[:, g, :, ns:ns + 512],
                            start=(g == 0), stop=(g == 2),
                            perf_mode=PM.DoubleRow,
                        )
                # copy on ACT: it is idle until the first scores anyway
                nc.scalar.activation(
                    out_t[:, i_s, :], pt[:, 0:1024], AF.Identity,
                    bias=bias, scale=1.0,
                )
                return
            for ns in (0, 512):
                pt = stage_slot(f"pjs{t}_{i_s}_{ns}")
                for g in range(3):
                    nc.tensor.matmul(
                        pt[:, 0:512],
                        lhsT=w_sb[:, g, :, c0:c0 + 128],
                        rhs=xp[:, g, :, ns:ns + 512],
                        start=(g == 0), stop=(g == 2),
                        perf_mode=PM.DoubleRow,
                    )
                nc.vector.tensor_scalar(
                    out=out_t[:, i_s, ns:ns + 512], in0=pt[:, 0:512],
                    scalar1=bias, scalar2=None, op0=ALU.add,
                )

        def proj_v(m):
            # staged single-bank psums so V never blocks the score slots
            for c0, nn in ((0, 512), (512, 256)):
                pt = stage_slot(f"vps{m}_{c0}")
                for g in range(3):
                    nc.tensor.matmul(
                        pt[:, 0:nn],
                        lhsT=xp[:, g, :, m * P:(m + 1) * P],
                        rhs=W["v"][:, g, :, c0:c0 + nn],
                        start=(g == 0), stop=(g == 2),
                        perf_mode=PM.DoubleRow,
                    )
                nc.vector.tensor_copy(V8[:, m, c0:c0 + nn], pt[:, 0:nn])

        # ---- attention emitters ----
        # per head: 6 exp units over 16 512-wide pieces (piece = j*2 + qhalf):
        # units of 3,3,3,3,3,1 pieces, strictly alternating psum slots
        # unit tables: (pieces, ...) with piece p = j*2 + qhalf.
        # normal head: 6 units over all 16 pieces, alternating psum slots.
        # offloaded head: units alternate (ACT, half-0 j-triple) and
        # (DVE-Schraudolph, half-1 j-triple).
        UNITS6 = ((0, 3), (3, 3), (6, 3), (9, 3), (12, 3), (15, 1))
        UNITSOFF = ((0, (0, 2, 4)), (1, (1, 3, 5)), (0, (6, 8, 10)),
                    (1, (7, 9, 11)), (0, (12, 14)), (1, (13, 15)))
        NUNIT = 6
        E8f = [E8[i].rearrange("p j s -> p (j s)") for i in range(NSLOT)]

        def emit_score_unit(h, u):
            t, gh = divmod(h, 4)
            if gh == 3:
                qt, kt, bp = QX, KX, 32 * t
            else:
                qt, kt, bp = QT[t], KT[t], 32 * gh
            off = h in OFF_HEADS
            if off:
                half1, pieces = UNITSOFF[u]
            else:
                p0, npc = UNITS6[u]
                half1, pieces = None, tuple(range(p0, p0 + npc))
            pt = psAB()
            for i, p in enumerate(pieces):
                j, half = divmod(p, 2)
                ns = half * 512
                nc.tensor.matmul(
                    pt[:, i * 512:(i + 1) * 512],
                    lhsT=kt[bp:bp + 32, :, j * P:(j + 1) * P],
                    rhs=qt[bp:bp + 32, :, ns:ns + 512],
                    start=True, stop=True,
                    perf_mode=PM.DoubleRow,
                )
            npc = len(pieces)
            j0 = pieces[0] // 2
            if off and half1:
                # DVE Schraudolph into the bf16 half-1 store
                dst = e8b_slot(h)[:, j0:j0 + npc, :]
                nc.vector.tensor_scalar(
                    out=dst.bitcast(mybir.dt.int16), in0=pt[:, 0:npc * 512],
                    scalar1=SC1, scalar2=SC2, op0=ALU.mult, op1=ALU.add,
                )
            elif off:
                # ACT exp of half-0 pieces: strided [j-triple, 0:512] dest
                nc.scalar.activation(
                    E8[h % NSLOT][:, j0:j0 + npc, 0:512], pt[:, 0:npc * 512],
                    AF.Exp, scale=0.125, bias=ebias[:, 0:1],
                )
            else:
                nc.scalar.activation(
                    E8f[h % NSLOT][:, pieces[0] * 512:
                                   (pieces[-1] + 1) * 512],
                    pt[:, 0:npc * 512],
                    AF.Exp, scale=0.125, bias=ebias[:, 0:1],
                )

        def emit_scores(h):
            for u in range(NUNIT):
                emit_score_unit(h, u)

        def emit_pr(h):
            prt = stage_slot(f"pr{h}")
            pr = prt[:, 0:8]
            off = h in OFF_HEADS
            et = E8[h % NSLOT]
            etb = e8b_slot(h) if off else None
            for m in range(8):
                if off and m >= 4:
                    for j in range(8):
                        nc.tensor.matmul(
                            pr[:, m:m + 1],
                            lhsT=etb[:, j, (m - 4) * P:(m - 3) * P],
                            rhs=halfcb[:, 0:1],
                            start=(j == 0), stop=(j == 7),
                        )
                else:
                    for jp in range(4):
                        nc.tensor.matmul(
                            pr[:, m:m + 1],
                            lhsT=et[:, jp * 2:jp * 2 + 2, m * P:(m + 1) * P],
                            rhs=halfc[:, :].rearrange("p (i o) -> p i o", o=1),
                            start=(jp == 0), stop=(jp == 3),
                            perf_mode=PM.DoubleRow,
                        )
            rinv = ap.tile([P, 8], F32, tag="rinv", bufs=2, name=f"rinv{h}")
            nc.vector.reciprocal(rinv, pr)
            return rinv

        def emit_ctx_m(h, rinv, m):
            # ping-pong across the two single-bank staging tags
            pc = stage_slot(f"pc{h}_{m}")
            if h in OFF_HEADS and m >= 4:
                etb = e8b_slot(h)
                for j in range(8):
                    nc.tensor.matmul(
                        pc[:, 0:64],
                        lhsT=etb[:, j, (m - 4) * P:(m - 3) * P],
                        rhs=V8[:, j, h * 64:(h + 1) * 64],
                        start=(j == 0), stop=(j == 7),
                    )
            else:
                et = E8[h % NSLOT]
                for jp in range(4):
                    nc.tensor.matmul(
                        pc[:, 0:64],
                        lhsT=et[:, jp * 2:jp * 2 + 2, m * P:(m + 1) * P],
                        rhs=V8[:, jp * 2:jp * 2 + 2, h * 64:(h + 1) * 64],
                        start=(jp == 0), stop=(jp == 3),
                        perf_mode=PM.DoubleRow,
                    )
            nc.vector.scalar_tensor_tensor(
                out=Y[m][:, h * 64:(h + 1) * 64], in0=pc[:, 0:64],
                scalar=rinv[:, m:m + 1], in1=xn2[m][:, h * 64:(h + 1) * 64],
                op0=ALU.mult, op1=ALU.add,
                accum_out=RS[m][:, h:h + 1],
            )

        def emit_ln(m):
            # var = E[y^2] - mu^2; final y = (y - mu) * rstd in one fused op.
            # sum(y^2) alternates ACT/DVE so the tail pipelines across engines.
            ysq = ap.tile([P, H], BF16, tag="ysq", bufs=2, name=f"ysq{m}")
            ss = ap.tile([P, 1], F32, tag="ss", bufs=3, name=f"ss{m}")
            if m % 2 == 0:
                nc.scalar.activation(ysq, Y[m], AF.Square,
                                     accum_out=ss[:, 0:1])
            else:
                nc.vector.scalar_tensor_tensor(
                    out=ysq, in0=Y[m], scalar=1.0, in1=Y[m],
                    op0=ALU.mult, op1=ALU.mult, accum_out=ss[:, 0:1],
                )
            nmu = ap.tile([P, 1], F32, tag="nmu", bufs=3, name=f"nmu{m}")
            nc.vector.tensor_reduce(out=nmu, in_=RS[m][:, 0:NH], axis=AX.X,
                                    op=ALU.add)
            nc.vector.tensor_scalar(out=nmu, in0=nmu, scalar1=-1.0 / H,
                                    scalar2=None, op0=ALU.mult)
            # bias = eps - mu^2  (per-partition), sd = sqrt(ss/H + bias)
            bia = ap.tile([P, 1], F32, tag="bia", bufs=3, name=f"bia{m}")
            nc.vector.scalar_tensor_tensor(
                out=bia, in0=nmu, scalar=-1.0, in1=nmu,
                op0=ALU.mult, op1=ALU.mult)
            nc.vector.tensor_scalar(out=bia, in0=bia, scalar1=1.0,
                                    scalar2=EPS, op0=ALU.mult, op1=ALU.add)
            sd = ap.tile([P, 1], F32, tag="sd", bufs=3, name=f"sd{m}")
            nc.scalar.activation(sd, ss, AF.Sqrt, scale=1.0 / H,
                                 bias=bia[:, 0:1])
            rstd = ap.tile([P, 1], F32, tag="rstd", bufs=3, name=f"rstd{m}")
            nc.vector.reciprocal(rstd, sd)
            nmr = ap.tile([P, 1], F32, tag="nmr", bufs=3, name=f"nmr{m}")
            nc.vector.scalar_tensor_tensor(
                out=nmr, in0=nmu, scalar=1.0, in1=rstd,
                op0=ALU.mult, op1=ALU.mult)
            nc.vector.tensor_scalar(out=Y[m], in0=Y[m], scalar1=rstd[:, 0:1],
                                    scalar2=nmr[:, 0:1], op0=ALU.mult,
                                    op1=ALU.add)
            if affine:
                nc.vector.tensor_tensor(out=Y[m], in0=Y[m], in1=gamb,
                                        op=ALU.mult)
                nc.vector.tensor_tensor(out=Y[m], in0=Y[m], in1=betb,
                                        op=ALU.add)
            nc.sync.dma_start(y_d[m * P:(m + 1) * P, :], Y[m])

        # ---- schedule: software-pipelined, scores run 4-5 heads ahead ----
        proj_qk(W["q"], bqp, QT[0], 0, 0)
        proj_qk(W["q"], bqp, QT[0], 0, 1)
        proj_qk(W["k"], bkp, KT[0], 0, 0)
        proj_qk(W["k"], bkp, KT[0], 0, 1)
        nc.sync.dma_start(QX[0:32, :, :], QT[0][96:P, :, :])
        nc.sync.dma_start(KX[0:32, :, :], KT[0][96:P, :, :])
        # heads 0-3 all live in tile 0: score them before t1/t2 projections
        emit_scores(0)
        emit_scores(1)
        for t in (1, 2):
            proj_qk(W["q"], bqp, QT[t], t, 0, staged=True)
            proj_qk(W["q"], bqp, QT[t], t, 1, staged=True)
            proj_qk(W["k"], bkp, KT[t], t, 0, staged=True)
            proj_qk(W["k"], bkp, KT[t], t, 1, staged=True)
            nc.sync.dma_start(QX[32 * t:32 * t + 32, :, :],
                              QT[t][96:P, :, :])
            nc.sync.dma_start(KX[32 * t:32 * t + 32, :, :],
                              KT[t][96:P, :, :])
            vq = [0, 1, 2, 3] if t == 1 else [4, 5, 6, 7]
            for u in range(NUNIT):
                emit_score_unit(t + 1, u)
                if u < 4:
                    proj_v(vq[u])
        # steady state: ctx(h) interleaved with scores(h+4) units
        AHEAD = 4
        for h in range(NH):
            ahead = h + AHEAD if h + AHEAD < NH else None
            rinv = emit_pr(h)
            for u, ms in ((0, (0, 1)), (1, (2, 3)), (2, (4, 5)), (3, (6, 7))):
                if ahead is not None:
                    emit_score_unit(ahead, u)
                for m in ms:
                    emit_ctx_m(h, rinv, m)
                    if h == NH - 1:
                        emit_ln(m)
            if ahead is not None:
                emit_score_unit(ahead, 4)
                emit_score_unit(ahead, 5)


def _get_nc(affine: bool):
    if affine not in _cache:
        _cache[affine] = _build(affine)
    return _cache[affine]


def _is_affine(inputs):
    gam = np.asarray(inputs["ln_gamma"], dtype=np.float32)
    bet = np.asarray(inputs["ln_beta"], dtype=np.float32)
    return not (np.all(gam == 1.0) and np.all(bet == 0.0))


def _perm_cols():
    # c' = t*256 + i*128 + m  ->  orig col 64*(4t + m//32) + 32*i + (m%32)
    cp = np.arange(H)
    t, r = np.divmod(cp, 256)
    i, m = np.divmod(r, 128)
    return 64 * (4 * t + m // 32) + 32 * i + (m % 32)


def _pair_rows(a):
    """[768, N] -> [128, 3, 2, N] with (p, g, i) <-> row 256g + 128i + p."""
    n = a.shape[1]
    return np.ascontiguousarray(
        a.reshape(3, 2, P, n).transpose(2, 0, 1, 3))


def make_in_maps(inputs):
    x = np.asarray(inputs["x"], dtype=np.float32)
    Wq = np.asarray(inputs["Wq"], dtype=np.float32)
    Wk = np.asarray(inputs["Wk"], dtype=np.float32)
    Wv = np.asarray(inputs["Wv"], dtype=np.float32)
    bq = np.asarray(inputs["bq"], dtype=np.float32)
    bk = np.asarray(inputs["bk"], dtype=np.float32)
    bv = np.asarray(inputs["bv"], dtype=np.float32)
    affine = _is_affine(inputs)

    perm = _perm_cols()
    wq8 = _pair_rows(Wq[:, perm]).astype(NPF8)
    wk8 = _pair_rows(Wk[:, perm]).astype(NPF8)
    wv8 = _pair_rows(Wv).astype(NPF8)

    # bias in output-partition layout: bqp[q, t*2+i] = bq[perm[t*256+i*128+q]]
    bqp = np.empty((P, 6), np.float32)
    bkp = np.empty((P, 6), np.float32)
    for t in range(3):
        for i in range(2):
            cols = perm[t * 256 + i * 128: t * 256 + i * 128 + P]
            bqp[:, t * 2 + i] = bq[cols]
            bkp[:, t * 2 + i] = bk[cols]

    in_maps = []
    for b in range(B):
        xb = x[b]
        im = {
            "xp": _pair_rows(np.ascontiguousarray(xb.T)).astype(NPF8),
            "xn2": (xb + 2.0 * bv).astype(NPBF16),
            "wq": wq8, "wk": wk8, "wv": wv8,
            "bqp": bqp, "bkp": bkp,
        }
        if affine:
            im["gamb"] = np.broadcast_to(
                np.asarray(inputs["ln_gamma"], np.float32), (P, H)
            ).astype(NPBF16)
            im["betb"] = np.broadcast_to(
                np.asarray(inputs["ln_beta"], np.float32), (P, H)
            ).astype(NPBF16)
        in_maps.append(im)
    return in_maps


def run(inputs, trace=False):
    nc = _get_nc(_is_affine(inputs))
    in_maps = make_in_maps(inputs)
    res = bass_utils.run_bass_kernel_spmd(
        nc, in_maps, core_ids=list(range(B)), trace=trace
    )
    out = np.stack(
        [r["y"].astype(np.float32) for r in res.results], axis=0)
    return out, res


def kernel(**inputs) -> np.ndarray:
    out, _ = run(inputs, trace=False)
    return out
